# revision 4
# baseline (speedup 1.0000x reference)
"""NeuralFingerprint GNN message passing on 8 TRN2 NeuronCores.

Sharding: each degree bucket split contiguously 8 ways (65536 rows/core,
bucket-major local order). conv0 runs on local rows gathering neighbor node
features from the replicated node table via one-offset-per-partition indirect
DMAs (accumulated in the DMA engine); edge features are host-permuted into
consumption order so they stream as linear DMAs. BN stats AllReduce; the
BN+ReLU'd conv0 output x is AllGathered so conv1 can gather arbitrary rows;
the final stage fuses BN+ReLU with the three softmax heads.
"""
import sys
import numpy as np

sys.path.insert(0, "/opt/trn_rl_repo")

N = 524288
E = 1433600
NODE, EDGE, H, OUT = 64, 16, 128, 128
BN_EPS = 1e-5
NCORES = 8
CNT = [8192, 65536, 131072, 196608, 98304, 24576]          # rows per degree d=0..5
PC = [c // NCORES for c in CNT]                            # per-core rows per degree
ROWS = sum(PC)                                             # 65536
GPD = [p // 512 for p in PC]                               # groups of 512 per degree
NG = sum(GPD)                                              # 128
START = [0, 8192, 73728, 204800, 401408, 499712]           # global bucket starts
LSTART = [0, 1024, 9216, 25600, 50176, 62464]              # local bucket starts
IDXW = 4 * sum(GPD[d] * d for d in range(6))               # 1400 packed idx cols
PCE = sum(PC[d] * d for d in range(6))                     # 179200 edge rows/core


def _group_meta():
    meta, off = [], 0
    for d in range(6):
        for _ in range(GPD[d]):
            meta.append((d, off))
            if d > 0:
                off += 4 * d
    return meta


def _build_nc():
    from concourse import bass, bacc, mybir
    import concourse.tile as tile
    from concourse.masks import make_identity

    f32, i32 = mybir.dt.float32, mybir.dt.int32
    AF = mybir.ActivationFunctionType
    ADD, BYP = mybir.AluOpType.add, mybir.AluOpType.bypass
    nc = bacc.Bacc("TRN2", target_bir_lowering=False)

    nf = nc.dram_tensor("nf", [N, NODE], f32, kind="ExternalInput")
    efp = nc.dram_tensor("efp", [PCE, EDGE], f32, kind="ExternalInput")
    nfT = nc.dram_tensor("nfT", [NODE, ROWS], f32, kind="ExternalInput")
    nidx0 = nc.dram_tensor("nidx0", [128, IDXW], i32, kind="ExternalInput")
    nidx1 = nc.dram_tensor("nidx1", [128, IDXW], i32, kind="ExternalInput")
    ws0T = nc.dram_tensor("ws0T", [NODE, H], f32, kind="ExternalInput")
    wd0nT = nc.dram_tensor("wd0nT", [NODE, 5 * H], f32, kind="ExternalInput")
    wd0eT = nc.dram_tensor("wd0eT", [EDGE, 5 * H], f32, kind="ExternalInput")
    ws1T = nc.dram_tensor("ws1T", [H, H], f32, kind="ExternalInput")
    wd1nT = nc.dram_tensor("wd1nT", [H, 5 * H], f32, kind="ExternalInput")
    wd1eT = nc.dram_tensor("wd1eT", [EDGE, 5 * H], f32, kind="ExternalInput")
    w0T = nc.dram_tensor("w0T", [NODE, OUT], f32, kind="ExternalInput")
    w1T = nc.dram_tensor("w1T", [H, OUT], f32, kind="ExternalInput")
    w2T = nc.dram_tensor("w2T", [H, OUT], f32, kind="ExternalInput")
    acc = nc.dram_tensor("acc", [ROWS, OUT], f32, kind="ExternalOutput")

    meta = _group_meta()
    RG = [list(range(NCORES))]

    with tile.TileContext(nc) as tc:
        with (
            tc.tile_pool(name="pers", bufs=1) as pers,
            tc.tile_pool(name="drp", bufs=1, space="DRAM") as drp,
            tc.tile_pool(name="iop", bufs=8) as iop,
            tc.tile_pool(name="gp", bufs=8) as gp,
            tc.tile_pool(name="rp", bufs=2) as rp,
            tc.tile_pool(name="spool", bufs=4) as spool,
            tc.tile_pool(name="pp", bufs=2, space="PSUM") as pp,
            tc.tile_pool(name="pt", bufs=4, space="PSUM") as pt,
        ):
            # DRAM staging (pool tiles so round-trips are dependency-tracked)
            a0T = drp.tile([128, ROWS], f32, name="a0T")
            xTd = drp.tile([128, ROWS], f32, name="xTd")
            a1T = drp.tile([128, ROWS], f32, name="a1T")
            x_rm = drp.tile([ROWS, H], f32, name="x_rm")
            x_full = drp.tile([N, H], f32, addr_space="Shared", name="x_full")
            cc0i = drp.tile([128, 2], f32, name="cc0i")
            cc0o = drp.tile([128, 2], f32, addr_space="Shared", name="cc0o")
            cc1i = drp.tile([128, 2], f32, name="cc1i")
            cc1o = drp.tile([128, 2], f32, addr_space="Shared", name="cc1o")

            # persistent SBUF (each name = own tag = own slot)
            w_s0 = pers.tile([NODE, H], f32, name="w_s0")
            nc.sync.dma_start(out=w_s0[:], in_=ws0T[:])
            w_d0n = pers.tile([NODE, 5 * H], f32, name="w_d0n")
            nc.sync.dma_start(out=w_d0n[:], in_=wd0nT[:])
            w_d0e = pers.tile([EDGE, 5 * H], f32, name="w_d0e")
            nc.sync.dma_start(out=w_d0e[:], in_=wd0eT[:])
            w_s1 = pers.tile([H, H], f32, name="w_s1")
            nc.sync.dma_start(out=w_s1[:], in_=ws1T[:])
            w_d1n = pers.tile([H, 5 * H], f32, name="w_d1n")
            nc.sync.dma_start(out=w_d1n[:], in_=wd1nT[:])
            w_d1e = pers.tile([EDGE, 5 * H], f32, name="w_d1e")
            nc.sync.dma_start(out=w_d1e[:], in_=wd1eT[:])
            w_0 = pers.tile([NODE, OUT], f32, name="w_0")
            nc.sync.dma_start(out=w_0[:], in_=w0T[:])
            w_1 = pers.tile([H, OUT], f32, name="w_1")
            nc.sync.dma_start(out=w_1[:], in_=w1T[:])
            w_2 = pers.tile([H, OUT], f32, name="w_2")
            nc.sync.dma_start(out=w_2[:], in_=w2T[:])
            ni0 = pers.tile([128, IDXW], i32, name="ni0")
            nc.sync.dma_start(out=ni0[:], in_=nidx0[:])
            ni1 = pers.tile([128, IDXW], i32, name="ni1")
            nc.sync.dma_start(out=ni1[:], in_=nidx1[:])
            ident = pers.tile([128, 128], f32, name="ident")
            make_identity(nc, ident[:])
            sum0 = pers.tile([128, NG], f32, name="sum0")
            ssq0 = pers.tile([128, NG], f32, name="ssq0")
            sum1 = pers.tile([128, NG], f32, name="sum1")
            ssq1 = pers.tile([128, NG], f32, name="ssq1")
            junk = pers.tile([128, NG], f32, name="junk")

            ebase = [0]

            def conv_stage(d, off, gg, selfw, selfrhs, wdn, wde, nidx, ntab, ND,
                           sum_t, ssq_t, aT):
                act = pp.tile([128, 512], f32, name="act", tag="mm")
                if d > 0:
                    rn = rp.tile([ND, 512], f32, name="rn", tag="rn")
                    re = rp.tile([EDGE, 512], f32, name="re", tag="re")
                    for c in range(4):
                        gsum = gp.tile([128, ND], f32, name="gsum", tag="gs")
                        for j in range(d):
                            col = off + c * d + j
                            nc.gpsimd.indirect_dma_start(
                                out=gsum[:], out_offset=None, in_=ntab[:],
                                in_offset=bass.IndirectOffsetOnAxis(
                                    ap=nidx[:, col:col + 1], axis=0),
                                compute_op=(BYP if j == 0 else ADD))
                        et = gp.tile([128, d * EDGE], f32, name="et", tag="et")
                        eb = ebase[0]
                        for j in range(d):
                            nc.sync.dma_start(
                                out=et[:, j * EDGE:(j + 1) * EDGE],
                                in_=efp[eb + j * 128:eb + (j + 1) * 128, :])
                        ebase[0] = eb + 128 * d
                        for j in range(1, d):
                            nc.vector.tensor_add(
                                out=et[:, 0:EDGE], in0=et[:, 0:EDGE],
                                in1=et[:, j * EDGE:(j + 1) * EDGE])
                        ptn = pt.tile([ND, 128], f32, name="ptn", tag="tp")
                        nc.tensor.transpose(ptn[:], gsum[:], ident[:])
                        nc.scalar.copy(out=rn[:, c * 128:(c + 1) * 128], in_=ptn[:])
                        pte = pt.tile([EDGE, 128], f32, name="pte", tag="tp")
                        nc.tensor.transpose(pte[:], et[:, 0:EDGE], ident[:])
                        nc.scalar.copy(out=re[:, c * 128:(c + 1) * 128], in_=pte[:])
                    nc.tensor.matmul(act[:], selfw[:], selfrhs[:], start=True, stop=False)
                    nc.tensor.matmul(act[:], wdn[:, (d - 1) * H:d * H], rn[:],
                                     start=False, stop=False)
                    nc.tensor.matmul(act[:], wde[:, (d - 1) * H:d * H], re[:],
                                     start=False, stop=True)
                else:
                    nc.tensor.matmul(act[:], selfw[:], selfrhs[:], start=True, stop=True)
                a_sb = iop.tile([128, 512], f32, name="a_sb", tag="io512")
                nc.scalar.activation(out=a_sb[:], in_=act[:], func=AF.Copy,
                                     accum_out=sum_t[:, gg:gg + 1])
                sq_sb = iop.tile([128, 512], f32, name="sq_sb", tag="io512")
                nc.scalar.activation(out=sq_sb[:], in_=act[:], func=AF.Square,
                                     accum_out=ssq_t[:, gg:gg + 1])
                nc.sync.dma_start(out=aT[:, gg * 512:(gg + 1) * 512], in_=a_sb[:])

            def bn_params(sum_t, ssq_t, cci, cco, tag):
                tot = pers.tile([128, 2], f32, name=f"tot{tag}")
                nc.scalar.activation(out=junk[:], in_=sum_t[:], func=AF.Copy,
                                     accum_out=tot[:, 0:1])
                nc.scalar.activation(out=junk[:], in_=ssq_t[:], func=AF.Copy,
                                     accum_out=tot[:, 1:2])
                nc.sync.dma_start(out=cci[:], in_=tot[:])
                nc.gpsimd.collective_compute(
                    "AllReduce", mybir.AluOpType.add, replica_groups=RG,
                    ins=[cci[:]], outs=[cco[:]])
                rt = pers.tile([128, 2], f32, name=f"rt{tag}")
                nc.sync.dma_start(out=rt[:], in_=cco[:])
                mean = pers.tile([128, 1], f32, name=f"mean{tag}")
                nc.vector.tensor_scalar_mul(out=mean[:], in0=rt[:, 0:1], scalar1=1.0 / N)
                var = pers.tile([128, 1], f32, name=f"var{tag}")
                nc.vector.tensor_scalar_mul(out=var[:], in0=rt[:, 1:2], scalar1=1.0 / N)
                m2 = pers.tile([128, 1], f32, name=f"m2{tag}")
                nc.scalar.square(out=m2[:], in_=mean[:])
                nc.vector.tensor_scalar_mul(out=m2[:], in0=m2[:], scalar1=-1.0)
                nc.vector.tensor_add(out=var[:], in0=var[:], in1=m2[:])
                nc.vector.tensor_scalar_add(out=var[:], in0=var[:], scalar1=BN_EPS)
                std = pers.tile([128, 1], f32, name=f"std{tag}")
                nc.scalar.sqrt(out=std[:], in_=var[:])
                istd = pers.tile([128, 1], f32, name=f"istd{tag}")
                nc.vector.reciprocal(out=istd[:], in_=std[:])
                bnb = pers.tile([128, 1], f32, name=f"bnb{tag}")
                nc.vector.tensor_scalar_mul(out=bnb[:], in0=mean[:], scalar1=-1.0)
                nc.vector.tensor_scalar_mul(out=bnb[:], in0=bnb[:], scalar1=istd[:, 0:1])
                return istd, bnb

            # ---- Stage A: conv0 pre-activations + stats ----
            for gg, (d, off) in enumerate(meta):
                nf_g = iop.tile([NODE, 512], f32, name="nf_g", tag="io512")
                nc.sync.dma_start(out=nf_g[:], in_=nfT[:, gg * 512:(gg + 1) * 512])
                conv_stage(d, off, gg, w_s0, nf_g, w_d0n, w_d0e, ni0, nf, NODE,
                           sum0, ssq0, a0T)

            istd0, bnb0 = bn_params(sum0, ssq0, cc0i, cc0o, "0")

            # ---- Stage A3: BN+ReLU -> xTd (col-major) and x_rm (row-major) ----
            for gg in range(NG):
                a_in = iop.tile([128, 512], f32, name="a_in", tag="io512")
                nc.sync.dma_start(out=a_in[:], in_=a0T[:, gg * 512:(gg + 1) * 512])
                xt = iop.tile([128, 512], f32, name="xt", tag="io512")
                nc.scalar.activation(out=xt[:], in_=a_in[:], func=AF.Relu,
                                     bias=bnb0[:, 0:1], scale=istd0[:, 0:1])
                nc.sync.dma_start(out=xTd[:, gg * 512:(gg + 1) * 512], in_=xt[:])
                for c in range(4):
                    pxt = pt.tile([128, 128], f32, name="pxt", tag="tp")
                    nc.tensor.transpose(pxt[:], xt[:, c * 128:(c + 1) * 128], ident[:])
                    xs = iop.tile([128, 128], f32, name="xs", tag="xs", bufs=4)
                    nc.scalar.copy(out=xs[:], in_=pxt[:])
                    nc.sync.dma_start(
                        out=x_rm[gg * 512 + c * 128:gg * 512 + (c + 1) * 128, :],
                        in_=xs[:])

            nc.gpsimd.collective_compute(
                "AllGather", mybir.AluOpType.bypass, replica_groups=RG,
                ins=[x_rm[:]], outs=[x_full[:]])

            # ---- Stage B: conv1 pre-activations + stats ----
            ebase[0] = 0
            for gg, (d, off) in enumerate(meta):
                xo = iop.tile([H, 512], f32, name="xo", tag="io512")
                nc.sync.dma_start(out=xo[:], in_=xTd[:, gg * 512:(gg + 1) * 512])
                conv_stage(d, off, gg, w_s1, xo, w_d1n, w_d1e, ni1, x_full, H,
                           sum1, ssq1, a1T)

            istd1, bnb1 = bn_params(sum1, ssq1, cc1i, cc1o, "1")

            # ---- Stage B3: BN+ReLU + 3 softmax heads ----
            for gg in range(NG):
                a_in2 = iop.tile([128, 512], f32, name="a_in2", tag="io512")
                nc.sync.dma_start(out=a_in2[:], in_=a1T[:, gg * 512:(gg + 1) * 512])
                yt = iop.tile([128, 512], f32, name="yt", tag="io512")
                nc.scalar.activation(out=yt[:], in_=a_in2[:], func=AF.Relu,
                                     bias=bnb1[:, 0:1], scale=istd1[:, 0:1])
                xt2 = iop.tile([128, 512], f32, name="xt2", tag="io512")
                nc.sync.dma_start(out=xt2[:], in_=xTd[:, gg * 512:(gg + 1) * 512])
                nfg = iop.tile([NODE, 512], f32, name="nfg", tag="io512")
                nc.sync.dma_start(out=nfg[:], in_=nfT[:, gg * 512:(gg + 1) * 512])
                for c in range(4):
                    ps3 = pp.tile([128, 384], f32, name="ps3", tag="mm")
                    nc.tensor.matmul(ps3[:, 0:128], nfg[:, c * 128:(c + 1) * 128],
                                     w_0[:], start=True, stop=True)
                    nc.tensor.matmul(ps3[:, 128:256], xt2[:, c * 128:(c + 1) * 128],
                                     w_1[:], start=True, stop=True)
                    nc.tensor.matmul(ps3[:, 256:384], yt[:, c * 128:(c + 1) * 128],
                                     w_2[:], start=True, stop=True)
                    esb = spool.tile([128, 384], f32, name="esb")
                    den = spool.tile([128, 3], f32, name="den")
                    for s in range(3):
                        nc.scalar.activation(
                            out=esb[:, s * 128:(s + 1) * 128],
                            in_=ps3[:, s * 128:(s + 1) * 128], func=AF.Exp,
                            accum_out=den[:, s:s + 1])
                    rec = spool.tile([128, 3], f32, name="rec")
                    nc.vector.reciprocal(out=rec[:], in_=den[:])
                    acc_sb = spool.tile([128, OUT], f32, name="acc_sb")
                    nc.vector.tensor_scalar_mul(out=acc_sb[:], in0=esb[:, 0:128],
                                                scalar1=rec[:, 0:1])
                    t1 = spool.tile([128, OUT], f32, name="t1")
                    nc.vector.tensor_scalar_mul(out=t1[:], in0=esb[:, 128:256],
                                                scalar1=rec[:, 1:2])
                    nc.vector.tensor_add(out=acc_sb[:], in0=acc_sb[:], in1=t1[:])
                    nc.vector.tensor_scalar_mul(out=t1[:], in0=esb[:, 256:384],
                                                scalar1=rec[:, 2:3])
                    nc.vector.tensor_add(out=acc_sb[:], in0=acc_sb[:], in1=t1[:])
                    nc.sync.dma_start(
                        out=acc[gg * 512 + c * 128:gg * 512 + (c + 1) * 128, :],
                        in_=acc_sb[:])
    return nc


class _Runner:
    def __init__(self, nc, n_cores):
        import jax
        from jax.experimental.shard_map import shard_map
        from jax.sharding import Mesh, PartitionSpec, NamedSharding
        from concourse import bass2jax, mybir
        from concourse.bass2jax import _bass_exec_p, install_neuronx_cc_hook

        install_neuronx_cc_hook()
        if hasattr(nc, "is_finalized") and not nc.is_finalized():
            nc.finalize()
        self.jax = jax
        self.n_cores = n_cores
        partition_name = nc.partition_id_tensor.name if nc.partition_id_tensor else None
        in_names, out_names, out_avals, zero_outs = [], [], [], []
        for alloc in nc.m.functions[0].allocations:
            if not isinstance(alloc, mybir.MemoryLocationSet):
                continue
            name = alloc.memorylocations[0].name
            if alloc.kind == "ExternalInput":
                if name != partition_name:
                    in_names.append(name)
            elif alloc.kind == "ExternalOutput":
                out_names.append(name)
                shape = tuple(alloc.tensor_shape)
                dtype = mybir.dt.np(alloc.dtype)
                out_avals.append(jax.core.ShapedArray(shape, dtype))
                zero_outs.append(np.zeros(shape, dtype))
        self.in_names, self.out_names, self.zero_outs = in_names, out_names, zero_outs
        all_in_names = in_names + out_names
        if partition_name:
            all_in_names = all_in_names + [partition_name]

        def _body(*args):
            operands = list(args)
            if partition_name:
                operands.append(bass2jax.partition_id_tensor())
            return tuple(
                _bass_exec_p.bind(
                    *operands,
                    out_avals=tuple(out_avals),
                    in_names=tuple(all_in_names),
                    out_names=tuple(out_names),
                    lowering_input_output_aliases=(),
                    sim_require_finite=True,
                    sim_require_nnan=True,
                    nc=nc,
                )
            )

        nio = len(in_names) + len(out_names)
        devices = jax.devices()[:n_cores]
        self.mesh = Mesh(np.asarray(devices), ("core",))
        self.sharding = NamedSharding(self.mesh, PartitionSpec("core"))
        self.fn = jax.jit(
            shard_map(
                _body,
                mesh=self.mesh,
                in_specs=(PartitionSpec("core"),) * nio,
                out_specs=(PartitionSpec("core"),) * len(out_names),
                check_rep=False,
            ),
            keep_unused=True,
        )
        self._dev_args = None

    def stage(self, in_maps):
        args = [
            np.concatenate(
                [np.asarray(in_maps[c][n]) for c in range(self.n_cores)], axis=0
            )
            for n in self.in_names
        ] + [np.concatenate([z] * self.n_cores, axis=0) for z in self.zero_outs]
        self._dev_args = [self.jax.device_put(a, self.sharding) for a in args]
        return self

    def call(self):
        out = self.fn(*self._dev_args)
        self.jax.block_until_ready(out)
        return out

    def results(self, out):
        res = []
        for c in range(self.n_cores):
            m = {}
            for i, name in enumerate(self.out_names):
                full = np.asarray(out[i])
                per = full.shape[0] // self.n_cores
                m[name] = full[c * per:(c + 1) * per]
            res.append(m)
        return res


def _own_rows(k):
    rows = np.empty(ROWS, np.int64)
    for b in range(6):
        rows[LSTART[b]:LSTART[b] + PC[b]] = START[b] + k * PC[b] + np.arange(PC[b])
    return rows


def _remap_to_ag(o):
    """Global node id -> position in AllGather output (core*ROWS + local)."""
    starts = np.array(START, np.int64)
    b = np.searchsorted(starts, o, side="right") - 1
    rel = o - starts[b]
    pcs = np.array(PC, np.int64)[b]
    core = rel // pcs
    loc = np.array(LSTART, np.int64)[b] + rel % pcs
    return (core * ROWS + loc).astype(np.int32)


def _pack_idx(arrs_by_d, k):
    """arrs_by_d[d] = [CNT[d], d] index array. Returns [128, IDXW] pack for core k.

    Column off(g) + c*d + j holds, at partition p, the j-th neighbor of local
    node g*512 + c*128 + p."""
    out = np.empty((128, IDXW), np.int32)
    off = 0
    for d in range(1, 6):
        a = arrs_by_d[d][k * PC[d]:(k + 1) * PC[d]]
        for g in range(GPD[d]):
            sub = a[g * 512:(g + 1) * 512]
            blk = sub.reshape(4, 128, d).transpose(1, 0, 2).reshape(128, 4 * d)
            out[:, off:off + 4 * d] = blk
            off += 4 * d
    return out


def _edge_perm(ne_by_d, k):
    """Edge row ids in device consumption order: (d, g, c, j, p)."""
    parts = []
    for d in range(1, 6):
        a = ne_by_d[d][k * PC[d]:(k + 1) * PC[d]]
        parts.append(
            a.reshape(GPD[d], 4, 128, d).transpose(0, 1, 3, 2).reshape(-1))
    return np.concatenate(parts)


_STATE = {}


def _get_runner():
    if "runner" not in _STATE:
        _STATE["runner"] = _Runner(_build_nc(), NCORES)
    return _STATE["runner"]


def kernel(**inputs):
    nf = np.ascontiguousarray(np.asarray(inputs["node_feat"], np.float32))
    ef = np.ascontiguousarray(np.asarray(inputs["edge_feat"], np.float32))
    nn = {d: np.asarray(inputs[f"nbr_node_d{d}"], np.int32) for d in range(1, 6)}
    ne = {d: np.asarray(inputs[f"nbr_edge_d{d}"], np.int32) for d in range(1, 6)}
    nn_ag = {d: _remap_to_ag(nn[d].astype(np.int64)) for d in range(1, 6)}

    wd0 = np.asarray(inputs["conv0_degW"], np.float32)
    wd1 = np.asarray(inputs["conv1_degW"], np.float32)
    shared = {
        "nf": nf,
        "ws0T": np.ascontiguousarray(np.asarray(inputs["conv0_selfW"], np.float32).T),
        "wd0nT": np.ascontiguousarray(
            np.concatenate([wd0[d][:, :NODE].T for d in range(5)], axis=1)),
        "wd0eT": np.ascontiguousarray(
            np.concatenate([wd0[d][:, NODE:].T for d in range(5)], axis=1)),
        "ws1T": np.ascontiguousarray(np.asarray(inputs["conv1_selfW"], np.float32).T),
        "wd1nT": np.ascontiguousarray(
            np.concatenate([wd1[d][:, :H].T for d in range(5)], axis=1)),
        "wd1eT": np.ascontiguousarray(
            np.concatenate([wd1[d][:, H:].T for d in range(5)], axis=1)),
        "w0T": np.ascontiguousarray(np.asarray(inputs["out_W0"], np.float32).T),
        "w1T": np.ascontiguousarray(np.asarray(inputs["out_W1"], np.float32).T),
        "w2T": np.ascontiguousarray(np.asarray(inputs["out_W2"], np.float32).T),
    }
    in_maps = []
    for k in range(NCORES):
        m = dict(shared)
        m["nfT"] = np.ascontiguousarray(nf[_own_rows(k)].T)
        m["efp"] = ef[_edge_perm(ne, k)]
        m["nidx0"] = _pack_idx(nn, k)
        m["nidx1"] = _pack_idx(nn_ag, k)
        in_maps.append(m)

    runner = _get_runner().stage(in_maps)
    out = runner.call()
    res = runner.results(out)
    acc_global = np.empty((N, OUT), np.float32)
    for k in range(NCORES):
        acc_global[_own_rows(k)] = res[k]["acc"]
    return acc_global[np.asarray(inputs["atom_index"])]


# revision 10
# speedup vs baseline: 1.0787x; 1.0787x over previous
"""NeuralFingerprint GNN message passing on 8 TRN2 NeuronCores.

Sharding: each degree bucket split contiguously 8 ways (65536 rows/core,
bucket-major local order). conv0 runs on local rows gathering neighbor node
features from the replicated node table via one-offset-per-partition indirect
DMAs (accumulated in the DMA engine); edge features are host-permuted into
consumption order so they stream as linear DMAs. BN stats AllReduce; the
BN+ReLU'd conv0 output x is AllGathered so conv1 can gather arbitrary rows;
the final stage fuses BN+ReLU with the three softmax heads.
"""
import sys
import numpy as np

sys.path.insert(0, "/opt/trn_rl_repo")

N = 524288
E = 1433600
NODE, EDGE, H, OUT = 64, 16, 128, 128
BN_EPS = 1e-5
NCORES = 8
CNT = [8192, 65536, 131072, 196608, 98304, 24576]          # rows per degree d=0..5
PC = [c // NCORES for c in CNT]                            # per-core rows per degree
ROWS = sum(PC)                                             # 65536
GPD = [p // 512 for p in PC]                               # groups of 512 per degree
NG = sum(GPD)                                              # 128
START = [0, 8192, 73728, 204800, 401408, 499712]           # global bucket starts
LSTART = [0, 1024, 9216, 25600, 50176, 62464]              # local bucket starts
IDXW = 4 * sum(GPD[d] * d for d in range(6))               # 1400 packed idx cols
PCE = sum(PC[d] * d for d in range(6))                     # 179200 edge rows/core


def _group_meta():
    meta, off = [], 0
    for d in range(6):
        for _ in range(GPD[d]):
            meta.append((d, off))
            if d > 0:
                off += 4 * d
    return meta


def _build_nc():
    from concourse import bass, bacc, mybir
    import concourse.tile as tile
    from concourse.masks import make_identity

    f32, i32, bf16 = mybir.dt.float32, mybir.dt.int32, mybir.dt.bfloat16
    AF = mybir.ActivationFunctionType
    ADD, BYP = mybir.AluOpType.add, mybir.AluOpType.bypass
    nc = bacc.Bacc("TRN2", target_bir_lowering=False)

    nf = nc.dram_tensor("nf", [N, NODE], f32, kind="ExternalInput")
    efp = nc.dram_tensor("efp", [PCE, EDGE], f32, kind="ExternalInput")
    nfT = nc.dram_tensor("nfT", [NODE, ROWS], f32, kind="ExternalInput")
    nidx0 = nc.dram_tensor("nidx0", [128, IDXW], i32, kind="ExternalInput")
    nidx1 = nc.dram_tensor("nidx1", [128, IDXW], i32, kind="ExternalInput")
    ws0T = nc.dram_tensor("ws0T", [NODE, H], f32, kind="ExternalInput")
    wd0nT = nc.dram_tensor("wd0nT", [NODE, 5 * H], f32, kind="ExternalInput")
    wd0eT = nc.dram_tensor("wd0eT", [EDGE, 5 * H], f32, kind="ExternalInput")
    ws1T = nc.dram_tensor("ws1T", [H, H], f32, kind="ExternalInput")
    wd1nT = nc.dram_tensor("wd1nT", [H, 5 * H], f32, kind="ExternalInput")
    wd1eT = nc.dram_tensor("wd1eT", [EDGE, 5 * H], f32, kind="ExternalInput")
    w0T = nc.dram_tensor("w0T", [NODE, OUT], f32, kind="ExternalInput")
    w1T = nc.dram_tensor("w1T", [H, OUT], f32, kind="ExternalInput")
    w2T = nc.dram_tensor("w2T", [H, OUT], f32, kind="ExternalInput")
    acc = nc.dram_tensor("acc", [ROWS, OUT], f32, kind="ExternalOutput")

    meta = _group_meta()
    RG = [list(range(NCORES))]

    with tile.TileContext(nc) as tc:
        with (
            tc.tile_pool(name="pers", bufs=1) as pers,
            tc.tile_pool(name="drp", bufs=1, space="DRAM") as drp,
            tc.tile_pool(name="iop", bufs=8) as iop,
            tc.tile_pool(name="gp", bufs=8) as gp,
            tc.tile_pool(name="rp", bufs=2) as rp,
            tc.tile_pool(name="spool", bufs=4) as spool,
            tc.tile_pool(name="pp", bufs=2, space="PSUM") as pp,
            tc.tile_pool(name="pt", bufs=4, space="PSUM") as pt,
        ):
            # DRAM staging (pool tiles so round-trips are dependency-tracked)
            a0T = drp.tile([128, ROWS], f32, name="a0T")
            xTd = drp.tile([128, ROWS], f32, name="xTd")
            a1T = drp.tile([128, ROWS], f32, name="a1T")
            x_rm = drp.tile([ROWS, H], bf16, name="x_rm")
            x_full = drp.tile([N, H], bf16, addr_space="Shared", name="x_full")
            cc0i = drp.tile([128, 2], f32, name="cc0i")
            cc0o = drp.tile([128, 2], f32, addr_space="Shared", name="cc0o")
            cc1i = drp.tile([128, 2], f32, name="cc1i")
            cc1o = drp.tile([128, 2], f32, addr_space="Shared", name="cc1o")

            # persistent SBUF (each name = own tag = own slot)
            w_s0 = pers.tile([NODE, H], f32, name="w_s0")
            nc.sync.dma_start(out=w_s0[:], in_=ws0T[:])
            w_d0n = pers.tile([NODE, 5 * H], f32, name="w_d0n")
            nc.sync.dma_start(out=w_d0n[:], in_=wd0nT[:])
            w_d0e = pers.tile([EDGE, 5 * H], f32, name="w_d0e")
            nc.sync.dma_start(out=w_d0e[:], in_=wd0eT[:])
            w_s1 = pers.tile([H, H], f32, name="w_s1")
            nc.sync.dma_start(out=w_s1[:], in_=ws1T[:])
            w_d1n = pers.tile([H, 5 * H], f32, name="w_d1n")
            nc.sync.dma_start(out=w_d1n[:], in_=wd1nT[:])
            w_d1e = pers.tile([EDGE, 5 * H], f32, name="w_d1e")
            nc.sync.dma_start(out=w_d1e[:], in_=wd1eT[:])
            w_0 = pers.tile([NODE, OUT], f32, name="w_0")
            nc.sync.dma_start(out=w_0[:], in_=w0T[:])
            w_1 = pers.tile([H, OUT], f32, name="w_1")
            nc.sync.dma_start(out=w_1[:], in_=w1T[:])
            w_2 = pers.tile([H, OUT], f32, name="w_2")
            nc.sync.dma_start(out=w_2[:], in_=w2T[:])
            ni0 = pers.tile([128, IDXW], i32, name="ni0")
            nc.sync.dma_start(out=ni0[:], in_=nidx0[:])
            ni1 = pers.tile([128, IDXW], i32, name="ni1")
            nc.sync.dma_start(out=ni1[:], in_=nidx1[:])
            ident = pers.tile([128, 128], f32, name="ident")
            make_identity(nc, ident[:])
            sum0 = pers.tile([128, NG], f32, name="sum0")
            ssq0 = pers.tile([128, NG], f32, name="ssq0")
            sum1 = pers.tile([128, NG], f32, name="sum1")
            ssq1 = pers.tile([128, NG], f32, name="ssq1")
            junk = pers.tile([128, NG], f32, name="junk")

            ebase = [0]

            def conv_stage(d, off, gg, selfw, selfrhs, wdn, wde, nidx, ntab, ND,
                           sum_t, ssq_t, aT, gbf=False):
                act = pp.tile([128, 512], f32, name="act", tag="mm")
                if d > 0:
                    rn = rp.tile([ND, 512], f32, name="rn", tag="rn")
                    re = rp.tile([EDGE, 512], f32, name="re", tag="re")
                    for c in range(4):
                        if gbf:
                            gt = gp.tile([128, d * ND], bf16, name="gt", tag="gs")
                            for j in range(d):
                                col = off + c * d + j
                                nc.gpsimd.indirect_dma_start(
                                    out=gt[:, j * ND:(j + 1) * ND],
                                    out_offset=None, in_=ntab[:],
                                    in_offset=bass.IndirectOffsetOnAxis(
                                        ap=nidx[:, col:col + 1], axis=0))
                            gsum = gp.tile([128, d * ND], f32, name="gf", tag="gf",
                                           bufs=4)
                            nc.scalar.copy(out=gsum[:], in_=gt[:])
                            for j in range(1, d):
                                nc.vector.tensor_add(
                                    out=gsum[:, 0:ND], in0=gsum[:, 0:ND],
                                    in1=gsum[:, j * ND:(j + 1) * ND])
                        else:
                            gsum = gp.tile([128, ND], f32, name="gsum", tag="gs")
                            for j in range(d):
                                col = off + c * d + j
                                nc.gpsimd.indirect_dma_start(
                                    out=gsum[:], out_offset=None, in_=ntab[:],
                                    in_offset=bass.IndirectOffsetOnAxis(
                                        ap=nidx[:, col:col + 1], axis=0),
                                    compute_op=(BYP if j == 0 else ADD))
                        et = gp.tile([128, d * EDGE], f32, name="et", tag="et")
                        eb = ebase[0]
                        for j in range(d):
                            nc.sync.dma_start(
                                out=et[:, j * EDGE:(j + 1) * EDGE],
                                in_=efp[eb + j * 128:eb + (j + 1) * 128, :])
                        ebase[0] = eb + 128 * d
                        for j in range(1, d):
                            nc.vector.tensor_add(
                                out=et[:, 0:EDGE], in0=et[:, 0:EDGE],
                                in1=et[:, j * EDGE:(j + 1) * EDGE])
                        ptn = pt.tile([ND, 128], f32, name="ptn", tag="tp")
                        nc.tensor.transpose(ptn[:], gsum[:, 0:ND], ident[:])
                        nc.scalar.copy(out=rn[:, c * 128:(c + 1) * 128], in_=ptn[:])
                        pte = pt.tile([EDGE, 128], f32, name="pte", tag="tp")
                        nc.tensor.transpose(pte[:], et[:, 0:EDGE], ident[:])
                        nc.scalar.copy(out=re[:, c * 128:(c + 1) * 128], in_=pte[:])
                    nc.tensor.matmul(act[:], selfw[:], selfrhs[:], start=True, stop=False)
                    nc.tensor.matmul(act[:], wdn[:, (d - 1) * H:d * H], rn[:],
                                     start=False, stop=False)
                    nc.tensor.matmul(act[:], wde[:, (d - 1) * H:d * H], re[:],
                                     start=False, stop=True)
                else:
                    nc.tensor.matmul(act[:], selfw[:], selfrhs[:], start=True, stop=True)
                a_sb = iop.tile([128, 512], f32, name="a_sb", tag="io512")
                nc.scalar.activation(out=a_sb[:], in_=act[:], func=AF.Copy,
                                     accum_out=sum_t[:, gg:gg + 1])
                sq_sb = iop.tile([128, 512], f32, name="sq_sb", tag="io512")
                nc.scalar.activation(out=sq_sb[:], in_=act[:], func=AF.Square,
                                     accum_out=ssq_t[:, gg:gg + 1])
                nc.sync.dma_start(out=aT[:, gg * 512:(gg + 1) * 512], in_=a_sb[:])

            def bn_params(sum_t, ssq_t, cci, cco, tag):
                tot = pers.tile([128, 2], f32, name=f"tot{tag}")
                nc.scalar.activation(out=junk[:], in_=sum_t[:], func=AF.Copy,
                                     accum_out=tot[:, 0:1])
                nc.scalar.activation(out=junk[:], in_=ssq_t[:], func=AF.Copy,
                                     accum_out=tot[:, 1:2])
                nc.sync.dma_start(out=cci[:], in_=tot[:])
                nc.gpsimd.collective_compute(
                    "AllReduce", mybir.AluOpType.add, replica_groups=RG,
                    ins=[cci[:]], outs=[cco[:]])
                rt = pers.tile([128, 2], f32, name=f"rt{tag}")
                nc.sync.dma_start(out=rt[:], in_=cco[:])
                mean = pers.tile([128, 1], f32, name=f"mean{tag}")
                nc.vector.tensor_scalar_mul(out=mean[:], in0=rt[:, 0:1], scalar1=1.0 / N)
                var = pers.tile([128, 1], f32, name=f"var{tag}")
                nc.vector.tensor_scalar_mul(out=var[:], in0=rt[:, 1:2], scalar1=1.0 / N)
                m2 = pers.tile([128, 1], f32, name=f"m2{tag}")
                nc.scalar.square(out=m2[:], in_=mean[:])
                nc.vector.tensor_scalar_mul(out=m2[:], in0=m2[:], scalar1=-1.0)
                nc.vector.tensor_add(out=var[:], in0=var[:], in1=m2[:])
                nc.vector.tensor_scalar_add(out=var[:], in0=var[:], scalar1=BN_EPS)
                std = pers.tile([128, 1], f32, name=f"std{tag}")
                nc.scalar.sqrt(out=std[:], in_=var[:])
                istd = pers.tile([128, 1], f32, name=f"istd{tag}")
                nc.vector.reciprocal(out=istd[:], in_=std[:])
                bnb = pers.tile([128, 1], f32, name=f"bnb{tag}")
                nc.vector.tensor_scalar_mul(out=bnb[:], in0=mean[:], scalar1=-1.0)
                nc.vector.tensor_scalar_mul(out=bnb[:], in0=bnb[:], scalar1=istd[:, 0:1])
                return istd, bnb

            # ---- Stage A: conv0 pre-activations + stats ----
            for gg, (d, off) in enumerate(meta):
                nf_g = iop.tile([NODE, 512], f32, name="nf_g", tag="io512")
                nc.sync.dma_start(out=nf_g[:], in_=nfT[:, gg * 512:(gg + 1) * 512])
                conv_stage(d, off, gg, w_s0, nf_g, w_d0n, w_d0e, ni0, nf, NODE,
                           sum0, ssq0, a0T)

            istd0, bnb0 = bn_params(sum0, ssq0, cc0i, cc0o, "0")

            # ---- Stage A3: BN+ReLU -> xTd (col-major) and x_rm (row-major) ----
            for gg in range(NG):
                a_in = iop.tile([128, 512], f32, name="a_in", tag="io512")
                nc.sync.dma_start(out=a_in[:], in_=a0T[:, gg * 512:(gg + 1) * 512])
                xt = iop.tile([128, 512], f32, name="xt", tag="io512")
                nc.scalar.activation(out=xt[:], in_=a_in[:], func=AF.Relu,
                                     bias=bnb0[:, 0:1], scale=istd0[:, 0:1])
                nc.sync.dma_start(out=xTd[:, gg * 512:(gg + 1) * 512], in_=xt[:])
                for c in range(4):
                    pxt = pt.tile([128, 128], f32, name="pxt", tag="tp")
                    nc.tensor.transpose(pxt[:], xt[:, c * 128:(c + 1) * 128], ident[:])
                    xs = iop.tile([128, 128], bf16, name="xs", tag="xs", bufs=4)
                    nc.scalar.copy(out=xs[:], in_=pxt[:])
                    nc.sync.dma_start(
                        out=x_rm[gg * 512 + c * 128:gg * 512 + (c + 1) * 128, :],
                        in_=xs[:])

            nc.gpsimd.collective_compute(
                "AllGather", mybir.AluOpType.bypass, replica_groups=RG,
                ins=[x_rm[:]], outs=[x_full[:]])

            # ---- Stage B: conv1 pre-activations + stats ----
            ebase[0] = 0
            for gg, (d, off) in enumerate(meta):
                xo = iop.tile([H, 512], f32, name="xo", tag="io512")
                nc.sync.dma_start(out=xo[:], in_=xTd[:, gg * 512:(gg + 1) * 512])
                conv_stage(d, off, gg, w_s1, xo, w_d1n, w_d1e, ni1, x_full, H,
                           sum1, ssq1, a1T, gbf=True)

            istd1, bnb1 = bn_params(sum1, ssq1, cc1i, cc1o, "1")

            # ---- Stage B3: BN+ReLU + 3 softmax heads ----
            for gg in range(NG):
                a_in2 = iop.tile([128, 512], f32, name="a_in2", tag="io512")
                nc.sync.dma_start(out=a_in2[:], in_=a1T[:, gg * 512:(gg + 1) * 512])
                yt = iop.tile([128, 512], f32, name="yt", tag="io512")
                nc.scalar.activation(out=yt[:], in_=a_in2[:], func=AF.Relu,
                                     bias=bnb1[:, 0:1], scale=istd1[:, 0:1])
                xt2 = iop.tile([128, 512], f32, name="xt2", tag="io512")
                nc.sync.dma_start(out=xt2[:], in_=xTd[:, gg * 512:(gg + 1) * 512])
                nfg = iop.tile([NODE, 512], f32, name="nfg", tag="io512")
                nc.sync.dma_start(out=nfg[:], in_=nfT[:, gg * 512:(gg + 1) * 512])
                for c in range(4):
                    ps3 = pp.tile([128, 384], f32, name="ps3", tag="mm")
                    nc.tensor.matmul(ps3[:, 0:128], nfg[:, c * 128:(c + 1) * 128],
                                     w_0[:], start=True, stop=True)
                    nc.tensor.matmul(ps3[:, 128:256], xt2[:, c * 128:(c + 1) * 128],
                                     w_1[:], start=True, stop=True)
                    nc.tensor.matmul(ps3[:, 256:384], yt[:, c * 128:(c + 1) * 128],
                                     w_2[:], start=True, stop=True)
                    esb = spool.tile([128, 384], f32, name="esb")
                    den = spool.tile([128, 3], f32, name="den")
                    for s in range(3):
                        nc.scalar.activation(
                            out=esb[:, s * 128:(s + 1) * 128],
                            in_=ps3[:, s * 128:(s + 1) * 128], func=AF.Exp,
                            accum_out=den[:, s:s + 1])
                    rec = spool.tile([128, 3], f32, name="rec")
                    nc.vector.reciprocal(out=rec[:], in_=den[:])
                    acc_sb = spool.tile([128, OUT], f32, name="acc_sb")
                    nc.vector.tensor_scalar_mul(out=acc_sb[:], in0=esb[:, 0:128],
                                                scalar1=rec[:, 0:1])
                    t1 = spool.tile([128, OUT], f32, name="t1")
                    nc.vector.tensor_scalar_mul(out=t1[:], in0=esb[:, 128:256],
                                                scalar1=rec[:, 1:2])
                    nc.vector.tensor_add(out=acc_sb[:], in0=acc_sb[:], in1=t1[:])
                    nc.vector.tensor_scalar_mul(out=t1[:], in0=esb[:, 256:384],
                                                scalar1=rec[:, 2:3])
                    nc.vector.tensor_add(out=acc_sb[:], in0=acc_sb[:], in1=t1[:])
                    nc.sync.dma_start(
                        out=acc[gg * 512 + c * 128:gg * 512 + (c + 1) * 128, :],
                        in_=acc_sb[:])
    return nc


class _Runner:
    def __init__(self, nc, n_cores):
        import jax
        from jax.experimental.shard_map import shard_map
        from jax.sharding import Mesh, PartitionSpec, NamedSharding
        from concourse import bass2jax, mybir
        from concourse.bass2jax import _bass_exec_p, install_neuronx_cc_hook

        install_neuronx_cc_hook()
        if hasattr(nc, "is_finalized") and not nc.is_finalized():
            nc.finalize()
        self.jax = jax
        self.n_cores = n_cores
        partition_name = nc.partition_id_tensor.name if nc.partition_id_tensor else None
        in_names, out_names, out_avals, zero_outs = [], [], [], []
        for alloc in nc.m.functions[0].allocations:
            if not isinstance(alloc, mybir.MemoryLocationSet):
                continue
            name = alloc.memorylocations[0].name
            if alloc.kind == "ExternalInput":
                if name != partition_name:
                    in_names.append(name)
            elif alloc.kind == "ExternalOutput":
                out_names.append(name)
                shape = tuple(alloc.tensor_shape)
                dtype = mybir.dt.np(alloc.dtype)
                out_avals.append(jax.core.ShapedArray(shape, dtype))
                zero_outs.append(np.zeros(shape, dtype))
        self.in_names, self.out_names, self.zero_outs = in_names, out_names, zero_outs
        all_in_names = in_names + out_names
        if partition_name:
            all_in_names = all_in_names + [partition_name]

        def _body(*args):
            operands = list(args)
            if partition_name:
                operands.append(bass2jax.partition_id_tensor())
            return tuple(
                _bass_exec_p.bind(
                    *operands,
                    out_avals=tuple(out_avals),
                    in_names=tuple(all_in_names),
                    out_names=tuple(out_names),
                    lowering_input_output_aliases=(),
                    sim_require_finite=True,
                    sim_require_nnan=True,
                    nc=nc,
                )
            )

        nio = len(in_names) + len(out_names)
        devices = jax.devices()[:n_cores]
        self.mesh = Mesh(np.asarray(devices), ("core",))
        self.sharding = NamedSharding(self.mesh, PartitionSpec("core"))
        self.fn = jax.jit(
            shard_map(
                _body,
                mesh=self.mesh,
                in_specs=(PartitionSpec("core"),) * nio,
                out_specs=(PartitionSpec("core"),) * len(out_names),
                check_rep=False,
            ),
            keep_unused=True,
        )
        self._dev_args = None

    def stage(self, in_maps):
        args = [
            np.concatenate(
                [np.asarray(in_maps[c][n]) for c in range(self.n_cores)], axis=0
            )
            for n in self.in_names
        ] + [np.concatenate([z] * self.n_cores, axis=0) for z in self.zero_outs]
        self._dev_args = [self.jax.device_put(a, self.sharding) for a in args]
        return self

    def call(self):
        out = self.fn(*self._dev_args)
        self.jax.block_until_ready(out)
        return out

    def results(self, out):
        res = []
        for c in range(self.n_cores):
            m = {}
            for i, name in enumerate(self.out_names):
                full = np.asarray(out[i])
                per = full.shape[0] // self.n_cores
                m[name] = full[c * per:(c + 1) * per]
            res.append(m)
        return res


def _own_rows(k):
    rows = np.empty(ROWS, np.int64)
    for b in range(6):
        rows[LSTART[b]:LSTART[b] + PC[b]] = START[b] + k * PC[b] + np.arange(PC[b])
    return rows


def _remap_to_ag(o):
    """Global node id -> position in AllGather output (core*ROWS + local)."""
    starts = np.array(START, np.int64)
    b = np.searchsorted(starts, o, side="right") - 1
    rel = o - starts[b]
    pcs = np.array(PC, np.int64)[b]
    core = rel // pcs
    loc = np.array(LSTART, np.int64)[b] + rel % pcs
    return (core * ROWS + loc).astype(np.int32)


def _pack_idx(arrs_by_d, k):
    """arrs_by_d[d] = [CNT[d], d] index array. Returns [128, IDXW] pack for core k.

    Column off(g) + c*d + j holds, at partition p, the j-th neighbor of local
    node g*512 + c*128 + p."""
    out = np.empty((128, IDXW), np.int32)
    off = 0
    for d in range(1, 6):
        a = arrs_by_d[d][k * PC[d]:(k + 1) * PC[d]]
        for g in range(GPD[d]):
            sub = a[g * 512:(g + 1) * 512]
            blk = sub.reshape(4, 128, d).transpose(1, 0, 2).reshape(128, 4 * d)
            out[:, off:off + 4 * d] = blk
            off += 4 * d
    return out


def _edge_perm(ne_by_d, k):
    """Edge row ids in device consumption order: (d, g, c, j, p)."""
    parts = []
    for d in range(1, 6):
        a = ne_by_d[d][k * PC[d]:(k + 1) * PC[d]]
        parts.append(
            a.reshape(GPD[d], 4, 128, d).transpose(0, 1, 3, 2).reshape(-1))
    return np.concatenate(parts)


_STATE = {}


def _get_runner():
    if "runner" not in _STATE:
        _STATE["runner"] = _Runner(_build_nc(), NCORES)
    return _STATE["runner"]


def kernel(**inputs):
    nf = np.ascontiguousarray(np.asarray(inputs["node_feat"], np.float32))
    ef = np.ascontiguousarray(np.asarray(inputs["edge_feat"], np.float32))
    nn = {d: np.asarray(inputs[f"nbr_node_d{d}"], np.int32) for d in range(1, 6)}
    ne = {d: np.asarray(inputs[f"nbr_edge_d{d}"], np.int32) for d in range(1, 6)}
    nn_ag = {d: _remap_to_ag(nn[d].astype(np.int64)) for d in range(1, 6)}

    wd0 = np.asarray(inputs["conv0_degW"], np.float32)
    wd1 = np.asarray(inputs["conv1_degW"], np.float32)
    shared = {
        "nf": nf,
        "ws0T": np.ascontiguousarray(np.asarray(inputs["conv0_selfW"], np.float32).T),
        "wd0nT": np.ascontiguousarray(
            np.concatenate([wd0[d][:, :NODE].T for d in range(5)], axis=1)),
        "wd0eT": np.ascontiguousarray(
            np.concatenate([wd0[d][:, NODE:].T for d in range(5)], axis=1)),
        "ws1T": np.ascontiguousarray(np.asarray(inputs["conv1_selfW"], np.float32).T),
        "wd1nT": np.ascontiguousarray(
            np.concatenate([wd1[d][:, :H].T for d in range(5)], axis=1)),
        "wd1eT": np.ascontiguousarray(
            np.concatenate([wd1[d][:, H:].T for d in range(5)], axis=1)),
        "w0T": np.ascontiguousarray(np.asarray(inputs["out_W0"], np.float32).T),
        "w1T": np.ascontiguousarray(np.asarray(inputs["out_W1"], np.float32).T),
        "w2T": np.ascontiguousarray(np.asarray(inputs["out_W2"], np.float32).T),
    }
    in_maps = []
    for k in range(NCORES):
        m = dict(shared)
        m["nfT"] = np.ascontiguousarray(nf[_own_rows(k)].T)
        m["efp"] = ef[_edge_perm(ne, k)]
        m["nidx0"] = _pack_idx(nn, k)
        m["nidx1"] = _pack_idx(nn_ag, k)
        in_maps.append(m)

    runner = _get_runner().stage(in_maps)
    out = runner.call()
    res = runner.results(out)
    acc_global = np.empty((N, OUT), np.float32)
    for k in range(NCORES):
        acc_global[_own_rows(k)] = res[k]["acc"]
    return acc_global[np.asarray(inputs["atom_index"])]


# revision 19
# speedup vs baseline: 1.4151x; 1.3119x over previous
"""NeuralFingerprint GNN message passing on 8 TRN2 NeuronCores.

Sharding: each degree bucket split contiguously 8 ways (65536 rows/core,
bucket-major local order). conv0 runs on local rows gathering neighbor node
features from the replicated node table via one-offset-per-partition indirect
DMAs (accumulated in the DMA engine); edge features are host-permuted into
consumption order so they stream as linear DMAs. BN stats AllReduce; the
BN+ReLU'd conv0 output x is AllGathered so conv1 can gather arbitrary rows;
the final stage fuses BN+ReLU with the three softmax heads.
"""
import sys
import numpy as np

sys.path.insert(0, "/opt/trn_rl_repo")

N = 524288
E = 1433600
NODE, EDGE, H, OUT = 64, 16, 128, 128
BN_EPS = 1e-5
NCORES = 8
CNT = [8192, 65536, 131072, 196608, 98304, 24576]          # rows per degree d=0..5
PC = [c // NCORES for c in CNT]                            # per-core rows per degree
ROWS = sum(PC)                                             # 65536
GPD = [p // 512 for p in PC]                               # groups of 512 per degree
NG = sum(GPD)                                              # 128
START = [0, 8192, 73728, 204800, 401408, 499712]           # global bucket starts
LSTART = [0, 1024, 9216, 25600, 50176, 62464]              # local bucket starts
IDXW = 4 * sum(GPD[d] * d for d in range(6))               # 1400 packed idx cols
PCE = sum(PC[d] * d for d in range(6))                     # 179200 edge rows/core


def _group_meta():
    meta, off = [], 0
    for d in range(6):
        for _ in range(GPD[d]):
            meta.append((d, off))
            if d > 0:
                off += 4 * d
    return meta


def _build_nc():
    from concourse import bass, bacc, mybir
    import concourse.tile as tile
    from concourse.masks import make_identity

    f32, i32, bf16 = mybir.dt.float32, mybir.dt.int32, mybir.dt.bfloat16
    AF = mybir.ActivationFunctionType
    ADD, BYP = mybir.AluOpType.add, mybir.AluOpType.bypass
    nc = bacc.Bacc("TRN2", target_bir_lowering=False)

    nf = nc.dram_tensor("nf", [N, NODE], bf16, kind="ExternalInput")
    efp = nc.dram_tensor("efp", [PCE, EDGE], f32, kind="ExternalInput")
    nfT = nc.dram_tensor("nfT", [NODE, ROWS], f32, kind="ExternalInput")
    nidx0 = nc.dram_tensor("nidx0", [128, IDXW], i32, kind="ExternalInput")
    nidx1 = nc.dram_tensor("nidx1", [128, IDXW], i32, kind="ExternalInput")
    ws0T = nc.dram_tensor("ws0T", [NODE, H], f32, kind="ExternalInput")
    wd0nT = nc.dram_tensor("wd0nT", [NODE, 5 * H], f32, kind="ExternalInput")
    wd0eT = nc.dram_tensor("wd0eT", [EDGE, 5 * H], f32, kind="ExternalInput")
    ws1T = nc.dram_tensor("ws1T", [H, H], f32, kind="ExternalInput")
    wd1nT = nc.dram_tensor("wd1nT", [H, 5 * H], f32, kind="ExternalInput")
    wd1eT = nc.dram_tensor("wd1eT", [EDGE, 5 * H], f32, kind="ExternalInput")
    w0T = nc.dram_tensor("w0T", [NODE, OUT], f32, kind="ExternalInput")
    w1T = nc.dram_tensor("w1T", [H, OUT], f32, kind="ExternalInput")
    w2T = nc.dram_tensor("w2T", [H, OUT], f32, kind="ExternalInput")
    acc = nc.dram_tensor("acc", [ROWS, OUT], f32, kind="ExternalOutput")

    meta = _group_meta()
    RG = [list(range(NCORES))]

    with tile.TileContext(nc) as tc:
        with (
            tc.tile_pool(name="pers", bufs=1) as pers,
            tc.tile_pool(name="drp", bufs=1, space="DRAM") as drp,
            tc.tile_pool(name="iop", bufs=8) as iop,
            tc.tile_pool(name="gp", bufs=8) as gp,
            tc.tile_pool(name="rp", bufs=2) as rp,
            tc.tile_pool(name="spool", bufs=4) as spool,
            tc.tile_pool(name="pp", bufs=2, space="PSUM") as pp,
            tc.tile_pool(name="pt", bufs=4, space="PSUM") as pt,
        ):
            # DRAM staging (pool tiles so round-trips are dependency-tracked)
            a0T = drp.tile([128, ROWS], bf16, name="a0T")
            xTd = drp.tile([128, ROWS], bf16, name="xTd")
            a1T = drp.tile([128, ROWS], bf16, name="a1T")
            x_rm = drp.tile([ROWS, H], bf16, name="x_rm")
            x_full = drp.tile([N, H], bf16, addr_space="Shared", name="x_full")
            cc0i = drp.tile([128, 2], f32, name="cc0i")
            cc0o = drp.tile([128, 2], f32, addr_space="Shared", name="cc0o")
            cc1i = drp.tile([128, 2], f32, name="cc1i")
            cc1o = drp.tile([128, 2], f32, addr_space="Shared", name="cc1o")

            # persistent SBUF (each name = own tag = own slot)
            w_s0 = pers.tile([NODE, H], f32, name="w_s0")
            nc.sync.dma_start(out=w_s0[:], in_=ws0T[:])
            w_d0n = pers.tile([NODE, 5 * H], f32, name="w_d0n")
            nc.sync.dma_start(out=w_d0n[:], in_=wd0nT[:])
            w_d0e = pers.tile([EDGE, 5 * H], f32, name="w_d0e")
            nc.sync.dma_start(out=w_d0e[:], in_=wd0eT[:])
            w_s1 = pers.tile([H, H], f32, name="w_s1")
            nc.sync.dma_start(out=w_s1[:], in_=ws1T[:])
            w_d1n = pers.tile([H, 5 * H], f32, name="w_d1n")
            nc.sync.dma_start(out=w_d1n[:], in_=wd1nT[:])
            w_d1e = pers.tile([EDGE, 5 * H], f32, name="w_d1e")
            nc.sync.dma_start(out=w_d1e[:], in_=wd1eT[:])
            w_0 = pers.tile([NODE, OUT], f32, name="w_0")
            nc.sync.dma_start(out=w_0[:], in_=w0T[:])
            w_1 = pers.tile([H, OUT], f32, name="w_1")
            nc.sync.dma_start(out=w_1[:], in_=w1T[:])
            w_2 = pers.tile([H, OUT], f32, name="w_2")
            nc.sync.dma_start(out=w_2[:], in_=w2T[:])
            ni0 = pers.tile([128, IDXW], i32, name="ni0")
            nc.sync.dma_start(out=ni0[:], in_=nidx0[:])
            ni1 = pers.tile([128, IDXW], i32, name="ni1")
            nc.sync.dma_start(out=ni1[:], in_=nidx1[:])
            ident = pers.tile([128, 128], f32, name="ident")
            make_identity(nc, ident[:])
            sum0 = pers.tile([128, NG], f32, name="sum0")
            ssq0 = pers.tile([128, NG], f32, name="ssq0")
            sum1 = pers.tile([128, NG], f32, name="sum1")
            ssq1 = pers.tile([128, NG], f32, name="ssq1")
            junk = pers.tile([128, NG], f32, name="junk")

            ebase = [0]

            def conv_stage(d, off, gg, selfw, selfrhs, wdn, wde, nidx, ntab, ND,
                           sum_t, ssq_t, aT, gbf=False):
                act = pp.tile([128, 512], f32, name="act", tag="mm")
                if d > 0:
                    rn = rp.tile([ND, 512], f32, name="rn", tag="rn")
                    re = rp.tile([EDGE, 512], f32, name="re", tag="re")
                    for c in range(4):
                        if gbf:
                            gt = gp.tile([128, d * ND], bf16, name="gt", tag="gs")
                            for j in range(d):
                                col = off + c * d + j
                                nc.gpsimd.indirect_dma_start(
                                    out=gt[:, j * ND:(j + 1) * ND],
                                    out_offset=None, in_=ntab[:],
                                    in_offset=bass.IndirectOffsetOnAxis(
                                        ap=nidx[:, col:col + 1], axis=0))
                            gsum = gp.tile([128, d * ND], f32, name="gf", tag="gf",
                                           bufs=4)
                            nc.scalar.copy(out=gsum[:], in_=gt[:])
                            for j in range(1, d):
                                nc.vector.tensor_add(
                                    out=gsum[:, 0:ND], in0=gsum[:, 0:ND],
                                    in1=gsum[:, j * ND:(j + 1) * ND])
                        else:
                            gsum = gp.tile([128, ND], f32, name="gsum", tag="gs")
                            for j in range(d):
                                col = off + c * d + j
                                nc.gpsimd.indirect_dma_start(
                                    out=gsum[:], out_offset=None, in_=ntab[:],
                                    in_offset=bass.IndirectOffsetOnAxis(
                                        ap=nidx[:, col:col + 1], axis=0),
                                    compute_op=(BYP if j == 0 else ADD))
                        et = gp.tile([128, d * EDGE], f32, name="et", tag="et")
                        eb = ebase[0]
                        for j in range(d):
                            nc.sync.dma_start(
                                out=et[:, j * EDGE:(j + 1) * EDGE],
                                in_=efp[eb + j * 128:eb + (j + 1) * 128, :])
                        ebase[0] = eb + 128 * d
                        for j in range(1, d):
                            nc.vector.tensor_add(
                                out=et[:, 0:EDGE], in0=et[:, 0:EDGE],
                                in1=et[:, j * EDGE:(j + 1) * EDGE])
                        ptn = pt.tile([ND, 128], f32, name="ptn", tag="tp")
                        nc.tensor.transpose(ptn[:], gsum[:, 0:ND], ident[:])
                        nc.scalar.copy(out=rn[:, c * 128:(c + 1) * 128], in_=ptn[:])
                        pte = pt.tile([EDGE, 128], f32, name="pte", tag="tp")
                        nc.tensor.transpose(pte[:], et[:, 0:EDGE], ident[:])
                        nc.scalar.copy(out=re[:, c * 128:(c + 1) * 128], in_=pte[:])
                    nc.tensor.matmul(act[:], selfw[:], selfrhs[:], start=True, stop=False)
                    nc.tensor.matmul(act[:], wdn[:, (d - 1) * H:d * H], rn[:],
                                     start=False, stop=False)
                    nc.tensor.matmul(act[:], wde[:, (d - 1) * H:d * H], re[:],
                                     start=False, stop=True)
                else:
                    nc.tensor.matmul(act[:], selfw[:], selfrhs[:], start=True, stop=True)
                a_sb = iop.tile([128, 512], f32, name="a_sb", tag="io512")
                nc.scalar.activation(out=a_sb[:], in_=act[:], func=AF.Copy,
                                     accum_out=sum_t[:, gg:gg + 1])
                sq_sb = iop.tile([128, 512], f32, name="sq_sb", tag="io512")
                nc.scalar.activation(out=sq_sb[:], in_=act[:], func=AF.Square,
                                     accum_out=ssq_t[:, gg:gg + 1])
                ab = iop.tile([128, 512], bf16, name="ab", tag="iob", bufs=4)
                nc.scalar.copy(out=ab[:], in_=act[:])
                nc.sync.dma_start(out=aT[:, gg * 512:(gg + 1) * 512], in_=ab[:])

            def bn_params(sum_t, ssq_t, cci, cco, tag):
                tot = pers.tile([128, 2], f32, name=f"tot{tag}")
                nc.scalar.activation(out=junk[:], in_=sum_t[:], func=AF.Copy,
                                     accum_out=tot[:, 0:1])
                nc.scalar.activation(out=junk[:], in_=ssq_t[:], func=AF.Copy,
                                     accum_out=tot[:, 1:2])
                nc.sync.dma_start(out=cci[:], in_=tot[:])
                nc.gpsimd.collective_compute(
                    "AllReduce", mybir.AluOpType.add, replica_groups=RG,
                    ins=[cci[:]], outs=[cco[:]])
                rt = pers.tile([128, 2], f32, name=f"rt{tag}")
                nc.sync.dma_start(out=rt[:], in_=cco[:])
                mean = pers.tile([128, 1], f32, name=f"mean{tag}")
                nc.vector.tensor_scalar_mul(out=mean[:], in0=rt[:, 0:1], scalar1=1.0 / N)
                var = pers.tile([128, 1], f32, name=f"var{tag}")
                nc.vector.tensor_scalar_mul(out=var[:], in0=rt[:, 1:2], scalar1=1.0 / N)
                m2 = pers.tile([128, 1], f32, name=f"m2{tag}")
                nc.scalar.square(out=m2[:], in_=mean[:])
                nc.vector.tensor_scalar_mul(out=m2[:], in0=m2[:], scalar1=-1.0)
                nc.vector.tensor_add(out=var[:], in0=var[:], in1=m2[:])
                nc.vector.tensor_scalar_add(out=var[:], in0=var[:], scalar1=BN_EPS)
                std = pers.tile([128, 1], f32, name=f"std{tag}")
                nc.scalar.sqrt(out=std[:], in_=var[:])
                istd = pers.tile([128, 1], f32, name=f"istd{tag}")
                nc.vector.reciprocal(out=istd[:], in_=std[:])
                bnb = pers.tile([128, 1], f32, name=f"bnb{tag}")
                nc.vector.tensor_scalar_mul(out=bnb[:], in0=mean[:], scalar1=-1.0)
                nc.vector.tensor_scalar_mul(out=bnb[:], in0=bnb[:], scalar1=istd[:, 0:1])
                return istd, bnb

            # ---- Stage A: conv0 pre-activations + stats ----
            for gg, (d, off) in enumerate(meta):
                nf_g = iop.tile([NODE, 512], f32, name="nf_g", tag="io512")
                nc.sync.dma_start(out=nf_g[:], in_=nfT[:, gg * 512:(gg + 1) * 512])
                conv_stage(d, off, gg, w_s0, nf_g, w_d0n, w_d0e, ni0, nf, NODE,
                           sum0, ssq0, a0T, gbf=True)

            istd0, bnb0 = bn_params(sum0, ssq0, cc0i, cc0o, "0")

            # ---- Stage A3: BN+ReLU -> xTd (col-major) and x_rm (row-major) ----
            for gg in range(NG):
                a_in = iop.tile([128, 512], bf16, name="a_in", tag="iob", bufs=4)
                nc.sync.dma_start(out=a_in[:], in_=a0T[:, gg * 512:(gg + 1) * 512])
                xt = iop.tile([128, 512], f32, name="xt", tag="io512")
                nc.scalar.activation(out=xt[:], in_=a_in[:], func=AF.Relu,
                                     bias=bnb0[:, 0:1], scale=istd0[:, 0:1])
                xtb = iop.tile([128, 512], bf16, name="xtb", tag="iob", bufs=4)
                nc.scalar.copy(out=xtb[:], in_=xt[:])
                nc.sync.dma_start(out=xTd[:, gg * 512:(gg + 1) * 512], in_=xtb[:])
                for c in range(4):
                    pxt = pt.tile([128, 128], f32, name="pxt", tag="tp")
                    nc.tensor.transpose(pxt[:], xt[:, c * 128:(c + 1) * 128], ident[:])
                    xs = iop.tile([128, 128], bf16, name="xs", tag="xs", bufs=4)
                    nc.scalar.copy(out=xs[:], in_=pxt[:])
                    nc.sync.dma_start(
                        out=x_rm[gg * 512 + c * 128:gg * 512 + (c + 1) * 128, :],
                        in_=xs[:])

            nc.gpsimd.collective_compute(
                "AllGather", mybir.AluOpType.bypass, replica_groups=RG,
                ins=[x_rm[:]], outs=[x_full[:]])

            # ---- Stage B: conv1 pre-activations + stats ----
            ebase[0] = 0
            for gg, (d, off) in enumerate(meta):
                xob = iop.tile([H, 512], bf16, name="xob", tag="iob", bufs=4)
                nc.sync.dma_start(out=xob[:], in_=xTd[:, gg * 512:(gg + 1) * 512])
                xo = iop.tile([H, 512], f32, name="xo", tag="io512")
                nc.scalar.copy(out=xo[:], in_=xob[:])
                conv_stage(d, off, gg, w_s1, xo, w_d1n, w_d1e, ni1, x_full, H,
                           sum1, ssq1, a1T, gbf=True)

            istd1, bnb1 = bn_params(sum1, ssq1, cc1i, cc1o, "1")

            # ---- Stage B3: BN+ReLU + 3 softmax heads ----
            for gg in range(NG):
                a_in2 = iop.tile([128, 512], bf16, name="a_in2", tag="iob", bufs=4)
                nc.sync.dma_start(out=a_in2[:], in_=a1T[:, gg * 512:(gg + 1) * 512])
                yt = iop.tile([128, 512], f32, name="yt", tag="io512")
                nc.scalar.activation(out=yt[:], in_=a_in2[:], func=AF.Relu,
                                     bias=bnb1[:, 0:1], scale=istd1[:, 0:1])
                xt2b = iop.tile([128, 512], bf16, name="xt2b", tag="iob", bufs=4)
                nc.sync.dma_start(out=xt2b[:], in_=xTd[:, gg * 512:(gg + 1) * 512])
                xt2 = iop.tile([128, 512], f32, name="xt2", tag="io512")
                nc.scalar.copy(out=xt2[:], in_=xt2b[:])
                nfg = iop.tile([NODE, 512], f32, name="nfg", tag="io512")
                nc.sync.dma_start(out=nfg[:], in_=nfT[:, gg * 512:(gg + 1) * 512])
                for c in range(4):
                    ps3 = pp.tile([128, 384], f32, name="ps3", tag="mm")
                    nc.tensor.matmul(ps3[:, 0:128], nfg[:, c * 128:(c + 1) * 128],
                                     w_0[:], start=True, stop=True)
                    nc.tensor.matmul(ps3[:, 128:256], xt2[:, c * 128:(c + 1) * 128],
                                     w_1[:], start=True, stop=True)
                    nc.tensor.matmul(ps3[:, 256:384], yt[:, c * 128:(c + 1) * 128],
                                     w_2[:], start=True, stop=True)
                    esb = spool.tile([128, 384], f32, name="esb")
                    den = spool.tile([128, 3], f32, name="den")
                    for s in range(3):
                        nc.scalar.activation(
                            out=esb[:, s * 128:(s + 1) * 128],
                            in_=ps3[:, s * 128:(s + 1) * 128], func=AF.Exp,
                            accum_out=den[:, s:s + 1])
                    rec = spool.tile([128, 3], f32, name="rec")
                    nc.vector.reciprocal(out=rec[:], in_=den[:])
                    acc_sb = spool.tile([128, OUT], f32, name="acc_sb")
                    nc.vector.tensor_scalar_mul(out=acc_sb[:], in0=esb[:, 0:128],
                                                scalar1=rec[:, 0:1])
                    t1 = spool.tile([128, OUT], f32, name="t1")
                    nc.vector.tensor_scalar_mul(out=t1[:], in0=esb[:, 128:256],
                                                scalar1=rec[:, 1:2])
                    nc.vector.tensor_add(out=acc_sb[:], in0=acc_sb[:], in1=t1[:])
                    nc.vector.tensor_scalar_mul(out=t1[:], in0=esb[:, 256:384],
                                                scalar1=rec[:, 2:3])
                    nc.vector.tensor_add(out=acc_sb[:], in0=acc_sb[:], in1=t1[:])
                    nc.sync.dma_start(
                        out=acc[gg * 512 + c * 128:gg * 512 + (c + 1) * 128, :],
                        in_=acc_sb[:])
    return nc


class _Runner:
    def __init__(self, nc, n_cores):
        import jax
        from jax.experimental.shard_map import shard_map
        from jax.sharding import Mesh, PartitionSpec, NamedSharding
        from concourse import bass2jax, mybir
        from concourse.bass2jax import _bass_exec_p, install_neuronx_cc_hook

        install_neuronx_cc_hook()
        if hasattr(nc, "is_finalized") and not nc.is_finalized():
            nc.finalize()
        self.jax = jax
        self.n_cores = n_cores
        partition_name = nc.partition_id_tensor.name if nc.partition_id_tensor else None
        in_names, out_names, out_avals, zero_outs = [], [], [], []
        for alloc in nc.m.functions[0].allocations:
            if not isinstance(alloc, mybir.MemoryLocationSet):
                continue
            name = alloc.memorylocations[0].name
            if alloc.kind == "ExternalInput":
                if name != partition_name:
                    in_names.append(name)
            elif alloc.kind == "ExternalOutput":
                out_names.append(name)
                shape = tuple(alloc.tensor_shape)
                dtype = mybir.dt.np(alloc.dtype)
                out_avals.append(jax.core.ShapedArray(shape, dtype))
                zero_outs.append(np.zeros(shape, dtype))
        self.in_names, self.out_names, self.zero_outs = in_names, out_names, zero_outs
        all_in_names = in_names + out_names
        if partition_name:
            all_in_names = all_in_names + [partition_name]

        def _body(*args):
            operands = list(args)
            if partition_name:
                operands.append(bass2jax.partition_id_tensor())
            return tuple(
                _bass_exec_p.bind(
                    *operands,
                    out_avals=tuple(out_avals),
                    in_names=tuple(all_in_names),
                    out_names=tuple(out_names),
                    lowering_input_output_aliases=(),
                    sim_require_finite=True,
                    sim_require_nnan=True,
                    nc=nc,
                )
            )

        nio = len(in_names) + len(out_names)
        devices = jax.devices()[:n_cores]
        self.mesh = Mesh(np.asarray(devices), ("core",))
        self.sharding = NamedSharding(self.mesh, PartitionSpec("core"))
        self.fn = jax.jit(
            shard_map(
                _body,
                mesh=self.mesh,
                in_specs=(PartitionSpec("core"),) * nio,
                out_specs=(PartitionSpec("core"),) * len(out_names),
                check_rep=False,
            ),
            keep_unused=True,
        )
        self._dev_args = None

    def stage(self, in_maps):
        args = [
            np.concatenate(
                [np.asarray(in_maps[c][n]) for c in range(self.n_cores)], axis=0
            )
            for n in self.in_names
        ] + [np.concatenate([z] * self.n_cores, axis=0) for z in self.zero_outs]
        self._dev_args = [self.jax.device_put(a, self.sharding) for a in args]
        return self

    def call(self):
        out = self.fn(*self._dev_args)
        self.jax.block_until_ready(out)
        return out

    def results(self, out):
        res = []
        for c in range(self.n_cores):
            m = {}
            for i, name in enumerate(self.out_names):
                full = np.asarray(out[i])
                per = full.shape[0] // self.n_cores
                m[name] = full[c * per:(c + 1) * per]
            res.append(m)
        return res


def _own_rows(k):
    rows = np.empty(ROWS, np.int64)
    for b in range(6):
        rows[LSTART[b]:LSTART[b] + PC[b]] = START[b] + k * PC[b] + np.arange(PC[b])
    return rows


def _remap_to_ag(o):
    """Global node id -> position in AllGather output (core*ROWS + local)."""
    starts = np.array(START, np.int64)
    b = np.searchsorted(starts, o, side="right") - 1
    rel = o - starts[b]
    pcs = np.array(PC, np.int64)[b]
    core = rel // pcs
    loc = np.array(LSTART, np.int64)[b] + rel % pcs
    return (core * ROWS + loc).astype(np.int32)


def _pack_idx(arrs_by_d, k):
    """arrs_by_d[d] = [CNT[d], d] index array. Returns [128, IDXW] pack for core k.

    Column off(g) + c*d + j holds, at partition p, the j-th neighbor of local
    node g*512 + c*128 + p."""
    out = np.empty((128, IDXW), np.int32)
    off = 0
    for d in range(1, 6):
        a = arrs_by_d[d][k * PC[d]:(k + 1) * PC[d]]
        for g in range(GPD[d]):
            sub = a[g * 512:(g + 1) * 512]
            blk = sub.reshape(4, 128, d).transpose(1, 0, 2).reshape(128, 4 * d)
            out[:, off:off + 4 * d] = blk
            off += 4 * d
    return out


def _edge_perm(ne_by_d, k):
    """Edge row ids in device consumption order: (d, g, c, j, p)."""
    parts = []
    for d in range(1, 6):
        a = ne_by_d[d][k * PC[d]:(k + 1) * PC[d]]
        parts.append(
            a.reshape(GPD[d], 4, 128, d).transpose(0, 1, 3, 2).reshape(-1))
    return np.concatenate(parts)


_STATE = {}


def _get_runner():
    if "runner" not in _STATE:
        _STATE["runner"] = _Runner(_build_nc(), NCORES)
    return _STATE["runner"]


def kernel(**inputs):
    nf = np.ascontiguousarray(np.asarray(inputs["node_feat"], np.float32))
    ef = np.ascontiguousarray(np.asarray(inputs["edge_feat"], np.float32))
    nn = {d: np.asarray(inputs[f"nbr_node_d{d}"], np.int32) for d in range(1, 6)}
    ne = {d: np.asarray(inputs[f"nbr_edge_d{d}"], np.int32) for d in range(1, 6)}
    nn_ag = {d: _remap_to_ag(nn[d].astype(np.int64)) for d in range(1, 6)}

    wd0 = np.asarray(inputs["conv0_degW"], np.float32)
    wd1 = np.asarray(inputs["conv1_degW"], np.float32)
    import ml_dtypes
    shared = {
        "nf": nf.astype(ml_dtypes.bfloat16),
        "ws0T": np.ascontiguousarray(np.asarray(inputs["conv0_selfW"], np.float32).T),
        "wd0nT": np.ascontiguousarray(
            np.concatenate([wd0[d][:, :NODE].T for d in range(5)], axis=1)),
        "wd0eT": np.ascontiguousarray(
            np.concatenate([wd0[d][:, NODE:].T for d in range(5)], axis=1)),
        "ws1T": np.ascontiguousarray(np.asarray(inputs["conv1_selfW"], np.float32).T),
        "wd1nT": np.ascontiguousarray(
            np.concatenate([wd1[d][:, :H].T for d in range(5)], axis=1)),
        "wd1eT": np.ascontiguousarray(
            np.concatenate([wd1[d][:, H:].T for d in range(5)], axis=1)),
        "w0T": np.ascontiguousarray(np.asarray(inputs["out_W0"], np.float32).T),
        "w1T": np.ascontiguousarray(np.asarray(inputs["out_W1"], np.float32).T),
        "w2T": np.ascontiguousarray(np.asarray(inputs["out_W2"], np.float32).T),
    }
    in_maps = []
    for k in range(NCORES):
        m = dict(shared)
        m["nfT"] = np.ascontiguousarray(nf[_own_rows(k)].T)
        m["efp"] = ef[_edge_perm(ne, k)]
        m["nidx0"] = _pack_idx(nn, k)
        m["nidx1"] = _pack_idx(nn_ag, k)
        in_maps.append(m)

    runner = _get_runner().stage(in_maps)
    out = runner.call()
    res = runner.results(out)
    acc_global = np.empty((N, OUT), np.float32)
    for k in range(NCORES):
        acc_global[_own_rows(k)] = res[k]["acc"]
    return acc_global[np.asarray(inputs["atom_index"])]
